# revision 2
# baseline (speedup 1.0000x reference)
"""CentroidPool (knn argmin) Trainium2 kernel.

kernel(latent [131072,128] f32, coords [1024,128] f32) -> closest centroid
index per row, int32 [131072].

Strategy: data-parallel over rows across 8 NeuronCores. Each core computes,
per 128-row tile, scores T = 2*x@c.T - |c|^2 via f32r matmuls (rank-1 matmul
accumulate folds the -|c|^2 term), reduces them to 64 group-maxes (groups of
16 centroids) on the Vector engine, and extracts the winning group plus the
top-2 group maxes with max8/max_index. The host then resolves the winning
group's 16 candidates in fp64 and fully recomputes the rare rows whose
cross-group gap is below a threshold (covers f32r truncation noise and ties).
"""

from contextlib import ExitStack

import numpy as np

import concourse.bacc as bacc
import concourse.mybir as mybir
import concourse.tile as tile
from concourse.bass_utils import run_bass_kernel_spmd

N = 131072
D = 128
K = 1024
N_CORES = 8
ROWS_PER_CORE = N // N_CORES        # 16384
TILE_ROWS = 128
N_TILES = ROWS_PER_CORE // TILE_ROWS  # 128
CHUNK_TILES = 8
L = 16                               # centroids per group
G = K // L                           # 64 groups
THETA = 2e-2                         # cross-group gap flag threshold

F32 = mybir.dt.float32
F32R = mybir.dt.float32r
U32 = mybir.dt.uint32

_CACHE: dict = {}


def _build_program(n_tiles: int = N_TILES, input_tiles: int | None = None):
    nc = bacc.Bacc("TRN2", target_bir_lowering=False, debug=False,
                   num_devices=N_CORES)
    n_rows = (input_tiles or n_tiles) * TILE_ROWS

    lat_t = nc.dram_tensor("lat_t", [D, n_rows], F32R, kind="ExternalInput").ap()
    coords2t = nc.dram_tensor("coords2t", [D, K], F32R, kind="ExternalInput").ap()
    negc2 = nc.dram_tensor("negc2", [1, K], F32R, kind="ExternalInput").ap()
    ones1 = nc.dram_tensor("ones1", [1, TILE_ROWS], F32R, kind="ExternalInput").ap()
    gidx_out = nc.dram_tensor("gidx", [TILE_ROWS, n_tiles], U32,
                              kind="ExternalOutput").ap()
    mv_out = nc.dram_tensor("mv", [TILE_ROWS, 2 * n_tiles], F32,
                            kind="ExternalOutput").ap()

    with ExitStack() as ctx:
        tc = ctx.enter_context(tile.TileContext(nc))
        const_pool = ctx.enter_context(tc.tile_pool(name="const", bufs=1))
        stage_pool = ctx.enter_context(tc.tile_pool(name="stage", bufs=1))
        lchunk_pool = ctx.enter_context(tc.tile_pool(name="lchunk", bufs=3))
        psum_pool = ctx.enter_context(tc.tile_pool(name="psum", bufs=4,
                                                   space="PSUM"))
        gmax_pool = ctx.enter_context(tc.tile_pool(name="gmax", bufs=4))
        small_pool = ctx.enter_context(tc.tile_pool(name="small", bufs=4))

        c2t_sb = const_pool.tile([D, K], F32R)
        negc2_sb = const_pool.tile([1, K], F32R)
        ones_sb = const_pool.tile([1, TILE_ROWS], F32R)
        nc.sync.dma_start(c2t_sb[:], coords2t[:])
        nc.sync.dma_start(negc2_sb[:], negc2[:])
        nc.sync.dma_start(ones_sb[:], ones1[:])

        staging_g = stage_pool.tile([TILE_ROWS, n_tiles], U32)
        staging_mv = stage_pool.tile([TILE_ROWS, 2 * n_tiles], F32)

        n_chunks = (n_tiles + CHUNK_TILES - 1) // CHUNK_TILES
        for c in range(n_chunks):
            t0 = c * CHUNK_TILES
            t1 = min(t0 + CHUNK_TILES, n_tiles)
            rows = (t1 - t0) * TILE_ROWS
            lchunk = lchunk_pool.tile([D, CHUNK_TILES * TILE_ROWS], F32R,
                                      tag="lchunk")
            nc.sync.dma_start(lchunk[:, :rows],
                              lat_t[:, t0 * TILE_ROWS: t1 * TILE_ROWS])
            for r in range(t1 - t0):
                t = t0 + r
                lt = lchunk[:, r * TILE_ROWS:(r + 1) * TILE_ROWS]
                ps = psum_pool.tile([TILE_ROWS, K], F32, tag="ps")
                for h in range(2):
                    sl = slice(h * 512, (h + 1) * 512)
                    nc.tensor.matmul(ps[:, sl], lt, c2t_sb[:, sl],
                                     start=True, stop=False)
                    nc.tensor.matmul(ps[:, sl], ones_sb[:], negc2_sb[:, sl],
                                     start=False, stop=True)
                gmax = gmax_pool.tile([TILE_ROWS, G], F32, tag="gmax")
                nc.vector.tensor_reduce(
                    out=gmax[:],
                    in_=ps[:].rearrange("p (g l) -> p g l", l=L),
                    axis=mybir.AxisListType.X, op=mybir.AluOpType.max)
                top8 = small_pool.tile([TILE_ROWS, 8], F32, tag="top8")
                idx8 = small_pool.tile([TILE_ROWS, 8], U32, tag="idx8")
                nc.vector.max(top8[:], gmax[:])
                nc.vector.max_index(idx8[:], top8[:], gmax[:])
                nc.vector.tensor_copy(staging_g[:, t:t + 1], idx8[:, 0:1])
                nc.vector.tensor_copy(staging_mv[:, 2 * t:2 * t + 2],
                                      top8[:, 0:2])

        nc.sync.dma_start(gidx_out[:], staging_g[:])
        nc.sync.dma_start(mv_out[:], staging_mv[:])

    nc.compile()
    return nc


def _get_program():
    if "nc" not in _CACHE:
        _CACHE["nc"] = _build_program()
    return _CACHE["nc"]


def kernel(latent: np.ndarray, coords: np.ndarray) -> np.ndarray:
    latent = np.asarray(latent, dtype=np.float32)
    coords = np.asarray(coords, dtype=np.float32)
    assert latent.shape == (N, D) and coords.shape == (K, D)

    nc = _get_program()

    coords2t = np.ascontiguousarray(2.0 * coords.T)
    c2_64 = (coords.astype(np.float64) ** 2).sum(1)
    negc2 = (-c2_64).astype(np.float32)[None, :]
    ones1 = np.ones((1, TILE_ROWS), np.float32)

    in_maps = []
    for c in range(N_CORES):
        sl = slice(c * ROWS_PER_CORE, (c + 1) * ROWS_PER_CORE)
        in_maps.append({
            "lat_t": np.ascontiguousarray(latent[sl].T),
            "coords2t": coords2t,
            "negc2": negc2,
            "ones1": ones1,
        })

    res = run_bass_kernel_spmd(nc, in_maps, list(range(N_CORES)))

    # gidx [128, n_tiles]: row n = t*128 + p -> [p, t]
    gidx = np.concatenate(
        [res.results[c]["gidx"].T.reshape(-1) for c in range(N_CORES)])
    mv = np.concatenate(
        [res.results[c]["mv"].reshape(TILE_ROWS, N_TILES, 2)
         .transpose(1, 0, 2).reshape(-1, 2) for c in range(N_CORES)])

    return _host_refine(latent, coords, gidx.astype(np.int64), mv, c2_64)


def _host_refine(lat, coords, gidx, mv, c2):
    lat64 = lat.astype(np.float64)
    coords64 = coords.astype(np.float64)
    cg = coords64.reshape(G, L, D)
    c2g = c2.reshape(G, L)
    out = np.empty(N, np.int64)
    chunk = 16384
    for s in range(0, N, chunk):
        e = min(s + chunk, N)
        g = gidx[s:e]
        sc = 2.0 * np.einsum('md,mld->ml', lat64[s:e], cg[g]) - c2g[g]
        out[s:e] = g * L + sc.argmax(1)
    flagged = np.flatnonzero(mv[:, 0] - mv[:, 1] < THETA)
    if flagged.size:
        sc = 2.0 * lat64[flagged] @ coords64.T - c2[None, :]
        out[flagged] = sc.argmax(1)
    return out.astype(np.int32)
